# revision 92
# baseline (speedup 1.0000x reference)
"""Two-layer GAT (single-head, PyG-style) + link predictor on 8 TRN2 NeuronCores.

Strategy (memory-regime):
  - Nodes sharded 8-way (6250/core, 49 windows of 128 dst nodes); edges
    (incl. self-loops) assigned to the core owning their dst and sorted by
    dst, so edge-softmax and the weighted scatter-sum are core-local.
  - The halo exchange runs on the host between launches: per-edge source
    feature rows are pre-expanded into a sequential fp16 stream
    [128, T, cols] (slot (p,t) = edge s%128, s//128 within its window), so
    the device does only large contiguous DMAs - no indirect gathers.
  - Segment softmax + weighted scatter run as one-hot matmuls on the PE:
        psum[d, :] += sum_e p_e * [dst_e == d] * stream[e, :]
    with a constant 1.0 column in each stream row accumulating the softmax
    denominator. exp() needs no segment-max shift (logits are O(6) and the
    shift cancels in the ratio). Sel matrices for a whole window are built
    with two stacked DVE ops using stride-0 3D broadcast APs.
  - Launch fusion: L2 = agg1 + proj2 (PE-transpose of the aggregated
    window then W2 matmul), with es2/ed2 and the link-predictor partial
    dots (W2@wl0, W2@wl1 columns) folded into the projection. L3 = agg2
    emitting only per-node d0/d1 dots; L4 combines sigmoid(d0[m0]+d1[m1]+b).
  - All floating-point math happens on device; the host does index-space
    work only (partitioning, sorting, expansion, fp16 table assembly).

Launches: L1 proj1 -> L2 agg1+proj2 -> L3 agg2+dots -> L4 combine.
"""
import sys
import time
import types

import numpy as np

# Environments differ in whether antenv.axon_hooks (the NTFF profile hook
# bridge) exists; install a shim wired to the boot helper when it's missing
# so trace=True works everywhere.
try:
    import antenv.axon_hooks  # noqa: F401
except ImportError:
    _hooks = types.ModuleType("antenv.axon_hooks")
    _hooks._hook = None
    _hooks.set_axon_ntff_profile_hook = lambda h: setattr(_hooks, "_hook", h)
    _hooks.get_axon_ntff_profile_hook = lambda: _hooks._hook
    sys.modules["antenv.axon_hooks"] = _hooks
    try:
        from trn_agent_boot.trn_boot import _ntff_profile_via_ctypes

        _hk = _ntff_profile_via_ctypes("/opt/axon/libaxon_pjrt.so")
        if _hk is not None:
            _hooks.set_axon_ntff_profile_hook(_hk)
    except Exception:
        pass

import concourse.bass as bass  # noqa: F401  (AP helpers)
import concourse.mybir as mybir
import concourse.tile as tile
from concourse import bacc
from concourse.bass_utils import run_bass_kernel_spmd

F32 = mybir.dt.float32
F16 = mybir.dt.float16
I32 = mybir.dt.int32

NCORES = 8
N, F_IN, H, C = 50000, 128, 256, 1
P = 10000
NS = N // NCORES            # 6250 nodes per shard
W = (NS + 127) // 128       # 49 windows per shard
NSP = W * 128               # 6272 padded slots
NEG = -1.0e30               # pad-edge sentinel (exp -> exactly 0)
PC = P // NCORES            # 1250 mask pairs per core
PT = (PC + 127) // 128      # 10 tiles of pairs

LAST_EXEC_NS = {}           # launch name -> exec_time_ns (filled per kernel() call)
_PROG_CACHE = {}
CMP_GPSIMD = False          # gpsimd can't lower broadcast APs; keep cmp on DVE


# ----------------------------------------------------------------- host prep
def _prep_graph(edge_index, wd):
    """Edges (incl. self-loops) partitioned by dst core, sorted by dst,
    window-padded to a common per-window tile count across cores. Windows
    are wd dst nodes wide; edge slot s within window w is
    (p, t) = (s % 128, wstart[w] + s // 128)."""
    nw = NSP // wd
    src = np.concatenate(
        [np.asarray(edge_index[0], np.int64), np.arange(N, dtype=np.int64)]
    )
    dst = np.concatenate(
        [np.asarray(edge_index[1], np.int64), np.arange(N, dtype=np.int64)]
    )
    core = dst // NS
    dstloc = dst - core * NS
    win = dstloc // wd

    order = np.lexsort((dstloc, core))
    src, core, dstloc, win = src[order], core[order], dstloc[order], win[order]

    cnt = np.zeros((NCORES, nw), np.int64)
    np.add.at(cnt, (core, win), 1)
    wt = np.maximum(1, (cnt + 127) // 128).max(axis=0)
    T = int(wt.sum())
    wstart = np.concatenate([[0], np.cumsum(wt)]).astype(np.int64)

    gid = core * nw + win
    first = np.ones(len(gid), bool)
    first[1:] = gid[1:] != gid[:-1]
    gstart = np.flatnonzero(first)
    startmap = np.zeros(NCORES * nw, np.int64)
    startmap[gid[gstart]] = gstart
    rank = np.arange(len(gid)) - startmap[gid]

    tt = wstart[win] + (rank >> 7)
    pp = rank & 127

    srcs = np.zeros((NCORES, 128, T), np.int32)
    dstg = np.zeros((NCORES, 128, T), np.int32)
    dstf = np.full((NCORES, 128, T), -1.0, np.float32)
    pad = np.ones((NCORES, 128, T), bool)
    srcs[core, pp, tt] = src
    dstg[core, pp, tt] = dstloc + core * NS
    dstf[core, pp, tt] = (dstloc - win * wd).astype(np.float32)
    pad[core, pp, tt] = False
    return dict(srcs=srcs, dstg=dstg, dstf=dstf, pad=pad, wt=wt, T=T, wd=wd)


def _edge_inputs(es, ed, g, c):
    """Per-slot es[src], ed[dst] (f32), with pad slots set to the exp->0
    sentinel."""
    esx = es[g["srcs"][c]].astype(np.float32)
    edx = ed[g["dstg"][c]].astype(np.float32)
    m = g["pad"][c]
    esx[m] = NEG
    edx[m] = 0.0
    return esx, edx


def _balance_perm(edge_index):
    """Permutation old-node -> new-node that balances (in-degree + 1) across
    the 392 (core, 128-window) bins, so per-window tile counts are near the
    mean instead of the max. Within each bin, snake-order by degree so the
    two 64-wide halves stay balanced too. Pure index-space work."""
    import heapq

    deg = np.bincount(
        np.asarray(edge_index[1], np.int64), minlength=N
    ) + 1
    caps = []
    for c in range(NCORES):
        for w in range(W):
            caps.append(min(128, NS - 128 * w))
    members = [[] for _ in caps]
    heap = [(0, b) for b in range(len(caps))]
    heapq.heapify(heap)
    order = np.argsort(-deg, kind="stable")
    for v in order:
        while True:
            load, b = heapq.heappop(heap)
            if len(members[b]) < caps[b]:
                break
        members[b].append(v)
        if len(members[b]) < caps[b]:
            heapq.heappush(heap, (load + int(deg[v]), b))
    perm = np.empty(N, np.int64)
    for b, mem in enumerate(members):
        c, w = divmod(b, W)
        # alternate degree-sorted members between the two 64-wide halves
        snake = mem[0::2] + mem[1::2][::-1]
        base = c * NS + 128 * w
        for i, v in enumerate(snake):
            perm[v] = base + i
    return perm


def _rep(v, n=128):
    return np.ascontiguousarray(
        np.broadcast_to(np.asarray(v, np.float32), (n, len(v)))
    )


def _tile_xT(x):
    """[N, 128] f32 features -> per-core [128, W*128] f16 transposed
    feature block for the L1 matmul lhsT slices."""
    out = np.zeros((NCORES, 128, W * 128), np.float16)
    for c in range(NCORES):
        xs = np.zeros((NSP, F_IN), np.float16)
        xs[:NS] = x[c * NS:(c + 1) * NS]
        out[c] = xs.T
    return out


# ------------------------------------------------------------- bass programs
def _build_p1(bias_zero):
    """L1: psum = xT.T @ [W1 | W1@a_s1 | W1@a_d1] per window; one fp16
    cast (+b1 fold) of the full 258-col psum per window -> h1e. Chunked
    input/output DMAs to stay off the HWDGE dispatch serialization. When
    b1 is all zero the psum->stage ops are plain casts split across the
    Act and DVE engines."""
    nc = bacc.Bacc(num_devices=NCORES)
    xT = nc.dram_tensor("xT", [128, W * 128], F16, kind="ExternalInput").ap()
    Wm = nc.dram_tensor("Wm", [F_IN, H], F16, kind="ExternalInput").ap()
    asr = nc.dram_tensor("asr", [128, H], F32, kind="ExternalInput").ap()
    adr = nc.dram_tensor("adr", [128, H], F32, kind="ExternalInput").ap()
    b1r = nc.dram_tensor("b1r", [128, H], F32, kind="ExternalInput").ap()
    h1e = nc.dram_tensor(
        "h1e", [128, W * (H + 2)], F16, kind="ExternalOutput"
    ).ap()

    with tile.TileContext(nc) as tc:
        with (
            tc.tile_pool(name="const", bufs=1) as cpool,
            tc.tile_pool(name="ps", bufs=4, space="PSUM") as pspool,
            tc.tile_pool(name="sc", bufs=2) as scpool,
        ):
            asb = cpool.tile([128, H], F32)
            nc.gpsimd.dma_start(out=asb[:], in_=asr[:])
            adb = cpool.tile([128, H], F32)
            nc.gpsimd.dma_start(out=adb[:], in_=adr[:])
            # b1 is folded into the message rows here: softmax weights sum
            # to one, so agg(h1 + b1) == agg(h1) + b1 downstream.
            b1x = cpool.tile([128, H + 2], F32)
            nc.vector.memset(b1x[:, H:H + 2], 0.0)
            nc.gpsimd.dma_start(out=b1x[:, 0:H], in_=b1r[:])
            xts = cpool.tile([128, W * 128], F16)
            nchunk = 7
            for k in range(nchunk):
                nc.sync.dma_start(
                    out=xts[:, k * W * 128 // nchunk:(k + 1) * W * 128 // nchunk],
                    in_=xT[:, k * W * 128 // nchunk:(k + 1) * W * 128 // nchunk],
                )
            waug = cpool.tile([128, H + 2], F16)
            nc.sync.dma_start(out=waug[:, 0:H], in_=Wm[:])
            w32 = cpool.tile([128, H], F32)
            nc.vector.tensor_copy(out=w32[:], in_=waug[:, 0:H])
            for j, vb in enumerate((asb, adb)):
                scr = scpool.tile([128, H], F32, tag="scr")
                nc.vector.tensor_tensor(
                    out=scr[:], in0=w32[:], in1=vb[:], op=mybir.AluOpType.mult
                )
                col = scpool.tile([128, 1], F32, tag="col")
                nc.vector.reduce_sum(
                    out=col[:], in_=scr[:], axis=mybir.AxisListType.X
                )
                nc.vector.tensor_copy(out=waug[:, H + j:H + j + 1], in_=col[:])

            stage = cpool.tile([128, W * (H + 2)], F16)
            WC = (W + nchunk - 1) // nchunk
            for w in range(W):
                ps = pspool.tile([128, H + 2], F32, space="PSUM")
                nc.tensor.matmul(
                    out=ps[:], lhsT=xts[:, 128 * w:128 * (w + 1)],
                    rhs=waug[:], start=True, stop=True,
                )
                dst = stage[:, w * (H + 2):(w + 1) * (H + 2)]
                if bias_zero and w % 2 == 0:
                    nc.scalar.copy(out=dst, in_=ps[:])
                elif bias_zero:
                    nc.vector.tensor_copy(out=dst, in_=ps[:])
                else:
                    nc.vector.tensor_tensor(
                        out=dst, in0=ps[:], in1=b1x[:],
                        op=mybir.AluOpType.add,
                    )
                if w % WC == WC - 1 or w == W - 1:
                    lo = (w // WC) * WC * (H + 2)
                    nc.sync.dma_start(
                        out=h1e[:, lo:(w + 1) * (H + 2)],
                        in_=stage[:, lo:(w + 1) * (H + 2)],
                    )
    nc.compile()
    return nc


def _build_agg(wt, cols, fuse_proj, wd):
    """Aggregation launch (one GAT layer).

    cols = stream row width (incl. trailing 1.0 denominator column and, for
    L3, the w0/w1 dot columns). Per window: one stream-slab DMA, a 2-op
    stacked sel build, wt[w] one-hot matmuls into psum, then either
      fuse_proj=True  (L2): normalize+bias+relu -> PE transpose -> W2aug
                      matmul -> h2e [NSP, 132] fp16 out
      fuse_proj=False (L3): d0/d1 = psum dot cols * rec + (b2.wl) -> d01.
    """
    T = int(sum(wt))
    npair = 128 // wd
    wtp = [
        sum(int(wt[npair * pw + s]) for s in range(npair))
        for pw in range(W)
    ]
    WTP = max(wtp)
    nc = bacc.Bacc(num_devices=NCORES)
    stream = nc.dram_tensor(
        "stream", [128, T * cols], F16, kind="ExternalInput"
    ).ap()
    dstf = nc.dram_tensor("dstf", [128, T], F16, kind="ExternalInput").ap()
    esx = nc.dram_tensor("esx", [128, T], F32, kind="ExternalInput").ap()
    edx = nc.dram_tensor("edx", [128, T], F32, kind="ExternalInput").ap()
    iota3 = nc.dram_tensor(
        "iota3", [128, wd, WTP], F16, kind="ExternalInput"
    ).ap()
    if fuse_proj:
        w2m = nc.dram_tensor("w2m", [H, F_IN], F16, kind="ExternalInput").ap()
        vr = [
            nc.dram_tensor(nm, [128, F_IN], F32, kind="ExternalInput").ap()
            for nm in ("as2r", "ad2r", "wl0r", "wl1r")
        ]
        idn = nc.dram_tensor("idn", [128, 128], F16, kind="ExternalInput").ap()
        h2e = nc.dram_tensor(
            "h2e", [128, W * (F_IN + 4)], F16, kind="ExternalOutput"
        ).ap()
    else:
        b2r = nc.dram_tensor("b2r", [128, F_IN], F32, kind="ExternalInput").ap()
        wl0r = nc.dram_tensor("wl0r", [128, F_IN], F32, kind="ExternalInput").ap()
        wl1r = nc.dram_tensor("wl1r", [128, F_IN], F32, kind="ExternalInput").ap()
        d01 = nc.dram_tensor("d01", [128, 2 * W], F32, kind="ExternalOutput").ap()

    with tile.TileContext(nc) as tc:
        with (
            tc.tile_pool(name="const", bufs=1) as cpool,
            tc.tile_pool(name="slab", bufs=8) as spool,
            tc.tile_pool(name="cmp", bufs=4) as cmppool,
            tc.tile_pool(name="sel", bufs=4) as selpool,
            tc.tile_pool(name="ep", bufs=4) as eppool,
            tc.tile_pool(name="o", bufs=3) as opool,
            tc.tile_pool(name="ps", bufs=4, space="PSUM") as pspool,
            tc.tile_pool(name="pt", bufs=2, space="PSUM") as ptpool,
            tc.tile_pool(name="p2", bufs=2, space="PSUM") as p2pool,
        ):
            # const inputs load via the idle gpsimd queue so the Sync queue
            # can start dispatching stream slabs immediately
            dsts = cpool.tile([128, T], F16)
            nc.gpsimd.dma_start(out=dsts[:], in_=dstf[:])
            esxs = cpool.tile([128, T], F32)
            nc.gpsimd.dma_start(out=esxs[:], in_=esx[:])
            edxs = cpool.tile([128, T], F32)
            nc.gpsimd.dma_start(out=edxs[:], in_=edx[:])
            io3 = cpool.tile([128, wd, WTP], F16)
            nc.gpsimd.dma_start(out=io3[:], in_=iota3[:])

            if fuse_proj:
                ids = cpool.tile([128, 128], F16)
                nc.gpsimd.dma_start(out=ids[:], in_=idn[:])
                vs = []
                for k, ap_ in enumerate(vr):
                    t_ = cpool.tile([128, F_IN], F32, tag=f"v{k}")
                    nc.gpsimd.dma_start(out=t_[:], in_=ap_[:])
                    vs.append(t_)
                w2aug = []
                for k in range(2):
                    wk = cpool.tile([128, F_IN + 4], F16, tag=f"w2a{k}")
                    nc.gpsimd.dma_start(
                        out=wk[:, 0:F_IN], in_=w2m[128 * k:128 * (k + 1), :]
                    )
                    wk32 = cpool.tile([128, F_IN], F32, tag=f"w232{k}")
                    nc.vector.tensor_copy(out=wk32[:], in_=wk[:, 0:F_IN])
                    for j, vb in enumerate(vs):
                        scr = cpool.tile([128, F_IN], F32, tag="fscr")
                        nc.vector.tensor_tensor(
                            out=scr[:], in0=wk32[:], in1=vb[:],
                            op=mybir.AluOpType.mult,
                        )
                        col = cpool.tile([128, 1], F32, tag="fcol")
                        nc.vector.reduce_sum(
                            out=col[:], in_=scr[:], axis=mybir.AxisListType.X
                        )
                        nc.vector.tensor_copy(
                            out=wk[:, F_IN + j:F_IN + j + 1], in_=col[:]
                        )
                    w2aug.append(wk)
            else:
                b2s = cpool.tile([128, F_IN], F32)
                nc.gpsimd.dma_start(out=b2s[:], in_=b2r[:])
                wl0s = cpool.tile([128, F_IN], F32)
                nc.gpsimd.dma_start(out=wl0s[:], in_=wl0r[:])
                wl1s = cpool.tile([128, F_IN], F32)
                nc.gpsimd.dma_start(out=wl1s[:], in_=wl1r[:])
                cc = cpool.tile([128, 2], F32)
                for j, vb in enumerate((wl0s, wl1s)):
                    scr = cpool.tile([128, F_IN], F32, tag="cscr")
                    nc.vector.tensor_tensor(
                        out=scr[:], in0=b2s[:], in1=vb[:],
                        op=mybir.AluOpType.mult,
                    )
                    nc.vector.reduce_sum(
                        out=cc[:, j:j + 1], in_=scr[:], axis=mybir.AxisListType.X
                    )
                d01s = cpool.tile([128, 2 * W], F32)

            # softmax numerators p = exp(leaky_relu(es+ed, 0.2)) in fp16
            lg = cpool.tile([128, T], F32)
            nc.vector.tensor_tensor(
                out=lg[:], in0=esxs[:], in1=edxs[:], op=mybir.AluOpType.add
            )
            lg2 = cpool.tile([128, T], F32)
            nc.vector.tensor_scalar_mul(out=lg2[:], in0=lg[:], scalar1=0.2)
            nc.vector.tensor_tensor(
                out=lg[:], in0=lg[:], in1=lg2[:], op=mybir.AluOpType.max
            )
            p16 = cpool.tile([128, T], F16)
            nc.scalar.activation(
                out=p16[:], in_=lg[:], func=mybir.ActivationFunctionType.Exp
            )

            if fuse_proj:
                stage = cpool.tile([128, W * (F_IN + 4)], F16)
            dcol = cols - 1 if fuse_proj else F_IN
            t0 = 0
            for pw in range(W):
                wtpg = wtp[pw]
                # one slab DMA per 128-node group of WD-wide windows
                slab = spool.tile([128, WTP * cols], F16)
                nc.sync.dma_start(
                    out=slab[:, 0:wtpg * cols],
                    in_=stream[:, t0 * cols:(t0 + wtpg) * cols],
                )
                # sel layout [slot, dst, tile]: per-(slot,tile) operands
                # broadcast on the MIDDLE dim, keeping innermost stride 1 so
                # the DVE 2x perf mode stays eligible. dstf is window-local,
                # so one op-pair covers every sub-window of the group.
                cmp3 = cmppool.tile([128, wd, WTP], F16)
                nc.vector.tensor_tensor(
                    out=cmp3[:, :, 0:wtpg], in0=io3[:, :, 0:wtpg],
                    in1=dsts[:, t0:t0 + wtpg].unsqueeze(1)
                        .broadcast_to([128, wd, wtpg]),
                    op=mybir.AluOpType.is_equal,
                )
                sel3 = selpool.tile([128, wd, WTP], F16)
                nc.vector.tensor_tensor(
                    out=sel3[:, :, 0:wtpg], in0=cmp3[:, :, 0:wtpg],
                    in1=p16[:, t0:t0 + wtpg].unsqueeze(1)
                        .broadcast_to([128, wd, wtpg]),
                    op=mybir.AluOpType.mult,
                )
                ps = pspool.tile([128, cols], F32, space="PSUM")
                tp = 0
                for sub in range(npair):
                    wtw = int(wt[npair * pw + sub])
                    for t in range(wtw):
                        nc.tensor.matmul(
                            out=ps[wd * sub:wd * (sub + 1), :],
                            lhsT=sel3[:, :, tp + t],
                            rhs=slab[:, (tp + t) * cols:(tp + t + 1) * cols],
                            start=(t == 0), stop=(t == wtw - 1),
                        )
                    tp += wtw
                w = pw
                rec = eppool.tile([128, 1], F32, tag="rec")
                nc.vector.reciprocal(rec[:], ps[:, dcol:dcol + 1])
                if fuse_proj:
                    # b1 is pre-folded into the message rows; normalize and
                    # rectify in one Act op: relu(agg * (1/den)).
                    h1r = eppool.tile([128, H], F16, tag="h1r")
                    nc.scalar.activation(
                        out=h1r[:], in_=ps[:, 0:H],
                        func=mybir.ActivationFunctionType.Relu,
                        scale=rec[:, :1],
                    )
                    xt = eppool.tile([128, H], F16, tag="xt")
                    psT = ptpool.tile([128, H], F16, space="PSUM")
                    for ck in range(2):
                        nc.tensor.transpose(
                            out=psT[:, 128 * ck:128 * (ck + 1)],
                            in_=h1r[:, 128 * ck:128 * (ck + 1)],
                            identity=ids[:],
                        )
                    nc.scalar.copy(out=xt[:], in_=psT[:])
                    ps2 = p2pool.tile([128, F_IN + 4], F32, space="PSUM")
                    nc.tensor.matmul(
                        out=ps2[:], lhsT=xt[:, 0:128], rhs=w2aug[0][:],
                        start=True, stop=False,
                    )
                    nc.tensor.matmul(
                        out=ps2[:], lhsT=xt[:, 128:256], rhs=w2aug[1][:],
                        start=False, stop=True,
                    )
                    nc.scalar.copy(
                        out=stage[:, w * (F_IN + 4):(w + 1) * (F_IN + 4)],
                        in_=ps2[:],
                    )
                    if w % 7 == 6 or w == W - 1:
                        lo = (w // 7) * 7 * (F_IN + 4)
                        nc.sync.dma_start(
                            out=h2e[:, lo:(w + 1) * (F_IN + 4)],
                            in_=stage[:, lo:(w + 1) * (F_IN + 4)],
                        )
                else:
                    nc.vector.scalar_tensor_tensor(
                        out=d01s[:, 2 * w:2 * w + 2],
                        in0=ps[:, F_IN + 1:F_IN + 3], scalar=rec[:, :1],
                        in1=cc[:], op0=mybir.AluOpType.mult,
                        op1=mybir.AluOpType.add,
                    )
                t0 += wtp[pw]
            if not fuse_proj:
                nc.sync.dma_start(out=d01[:], in_=d01s[:])
    nc.compile()
    return nc


def _build_comb():
    """L4: z = sigmoid(d0[m0] + d1[m1] + bl) for PC pairs per core."""
    nc = bacc.Bacc(num_devices=NCORES)
    d0x = nc.dram_tensor("d0x", [128, PT], F32, kind="ExternalInput").ap()
    d1x = nc.dram_tensor("d1x", [128, PT], F32, kind="ExternalInput").ap()
    blr = nc.dram_tensor("blr", [128, 1], F32, kind="ExternalInput").ap()
    z = nc.dram_tensor("z", [128, PT], F32, kind="ExternalOutput").ap()

    with tile.TileContext(nc) as tc:
        with tc.tile_pool(name="p", bufs=1) as pool:
            d0s = pool.tile([128, PT], F32)
            nc.sync.dma_start(out=d0s[:], in_=d0x[:])
            d1s = pool.tile([128, PT], F32)
            nc.sync.dma_start(out=d1s[:], in_=d1x[:])
            bls = pool.tile([128, 1], F32)
            nc.sync.dma_start(out=bls[:], in_=blr[:])
            ss = pool.tile([128, PT], F32)
            nc.vector.tensor_tensor(
                out=ss[:], in0=d0s[:], in1=d1s[:], op=mybir.AluOpType.add
            )
            zs = pool.tile([128, PT], F32)
            nc.scalar.activation(
                out=zs[:], in_=ss[:],
                func=mybir.ActivationFunctionType.Sigmoid, bias=bls[:, :1],
            )
            nc.sync.dma_start(out=z[:], in_=zs[:])
    nc.compile()
    return nc


def _run(name, nc, in_maps, trace=True):
    last = None
    for attempt in range(3):
        try:
            res = run_bass_kernel_spmd(
                nc, in_maps, core_ids=list(range(NCORES)),
                trace=trace and attempt < 2,
            )
            LAST_EXEC_NS[name] = res.exec_time_ns
            return res.results
        except Exception as e:  # wedged-device retry (clears on re-attempt)
            last = e
            time.sleep(5)
    raise last


# ------------------------------------------------------------------- kernel
def kernel(features, edge_index, mask, W1, a_src1, a_dst1, b1, W2, a_src2,
           a_dst2, b2, Wl, bl):
    features = np.asarray(features, np.float32)
    edge_index = np.asarray(edge_index, np.int32)
    mask = np.asarray(mask, np.int32)
    W1, W2, Wl = (np.asarray(a, np.float32) for a in (W1, W2, Wl))
    a_src1, a_dst1, b1 = (np.asarray(a, np.float32) for a in (a_src1, a_dst1, b1))
    a_src2, a_dst2, b2 = (np.asarray(a, np.float32) for a in (a_src2, a_dst2, b2))
    bl = np.asarray(bl, np.float32)

    # degree-balanced node relabeling (transparent: mask rows keep order)
    perm = _balance_perm(edge_index)
    features = features[np.argsort(perm)]
    edge_index = perm[edge_index].astype(np.int32)
    mask = perm[mask].astype(np.int32)

    g2 = _prep_graph(edge_index, 128)   # L2: wide windows (moving cols dominate)
    g3 = _prep_graph(edge_index, 64)    # L3: narrow windows (sel build dominates)

    def _iota3(g):
        wd = g["wd"]
        npair = 128 // wd
        wtp = g["wt"].reshape(-1, npair).sum(axis=1)
        return np.ascontiguousarray(np.broadcast_to(
            np.arange(wd, dtype=np.float16)[None, :, None],
            (128, wd, int(max(wtp))),
        ))

    iota2, iota3_ = _iota3(g2), _iota3(g3)
    idn = np.eye(128, dtype=np.float16)
    dstf2 = g2["dstf"].astype(np.float16)
    dstf3 = g3["dstf"].astype(np.float16)

    b1zero = not np.any(b1)
    key = (tuple(int(x) for x in g2["wt"]), tuple(int(x) for x in g3["wt"]),
           b1zero)
    if key not in _PROG_CACHE:
        _PROG_CACHE[key] = dict(
            p1=_build_p1(b1zero),
            l2=_build_agg(g2["wt"], H + 1, fuse_proj=True, wd=128),
            l3=_build_agg(g3["wt"], F_IN + 3, fuse_proj=False, wd=64),
            l4=_build_comb(),
        )
    progs = _PROG_CACHE[key]

    # ---- L1: h1e = [X@W1 | es1 | ed1] (sharded)
    xT = _tile_xT(features)
    W1h = W1.astype(np.float16)
    b1r = _rep(b1)
    r1 = _run("p1", progs["p1"], [
        dict(xT=xT[c], Wm=W1h, asr=_rep(a_src1), adr=_rep(a_dst1), b1r=b1r)
        for c in range(NCORES)
    ])
    H1 = np.concatenate([
        r1[c]["h1e"].reshape(128, W, H + 2).transpose(1, 0, 2)
        .reshape(NSP, H + 2)[:NS]
        for c in range(NCORES)
    ])  # [N, 258] f16
    es1 = H1[:, H].astype(np.float32)
    ed1 = H1[:, H + 1].astype(np.float32)
    table1 = np.empty((N, H + 1), np.float16)
    table1[:, :H] = H1[:, :H]
    table1[:, H] = 1.0

    # ---- L2: aggregate layer 1, project through W2aug
    as2r, ad2r = _rep(a_src2), _rep(a_dst2)
    wl0r, wl1r = _rep(Wl[:F_IN, 0]), _rep(Wl[F_IN:, 0])
    W2h = W2.astype(np.float16)
    ins2 = []
    for c in range(NCORES):
        esx, edx = _edge_inputs(es1, ed1, g2, c)
        strm = table1[g2["srcs"][c]].reshape(128, g2["T"] * (H + 1))
        ins2.append(dict(stream=strm, dstf=dstf2[c], esx=esx, edx=edx,
                         iota3=iota2, w2m=W2h, as2r=as2r, ad2r=ad2r,
                         wl0r=wl0r, wl1r=wl1r, idn=idn))
    r2 = _run("l2", progs["l2"], ins2)
    H2 = np.concatenate([
        r2[c]["h2e"].reshape(128, W, F_IN + 4).transpose(1, 0, 2)
        .reshape(NSP, F_IN + 4)[:NS]
        for c in range(NCORES)
    ])  # [N, 132] f16
    es2 = H2[:, F_IN].astype(np.float32)
    ed2 = H2[:, F_IN + 1].astype(np.float32)
    table2 = np.empty((N, F_IN + 3), np.float16)
    table2[:, :F_IN] = H2[:, :F_IN]
    table2[:, F_IN] = 1.0
    table2[:, F_IN + 1:F_IN + 3] = H2[:, F_IN + 2:F_IN + 4]

    # ---- L3: aggregate layer 2 -> per-node link dots d0, d1
    b2r = _rep(b2)
    ins3 = []
    for c in range(NCORES):
        esx, edx = _edge_inputs(es2, ed2, g3, c)
        strm = table2[g3["srcs"][c]].reshape(128, g3["T"] * (F_IN + 3))
        ins3.append(dict(stream=strm, dstf=dstf3[c], esx=esx, edx=edx,
                         iota3=iota3_, b2r=b2r, wl0r=wl0r, wl1r=wl1r))
    r3 = _run("l3", progs["l3"], ins3)
    d0g = np.concatenate(
        [r3[c]["d01"][:, 0::2].T.ravel()[:NS] for c in range(NCORES)]
    )
    d1g = np.concatenate(
        [r3[c]["d01"][:, 1::2].T.ravel()[:NS] for c in range(NCORES)]
    )

    # ---- L4: z = sigmoid(d0[m0] + d1[m1] + bl)
    mT = mask.T
    blr = np.full((128, 1), float(bl[0]), np.float32)
    s = np.arange(PC)
    ins4 = []
    for c in range(NCORES):
        d0x = np.zeros((128, PT), np.float32)
        d1x = np.zeros((128, PT), np.float32)
        d0x[s % 128, s // 128] = d0g[mT[0][c * PC:(c + 1) * PC]]
        d1x[s % 128, s // 128] = d1g[mT[1][c * PC:(c + 1) * PC]]
        ins4.append(dict(d0x=d0x, d1x=d1x, blr=blr))
    r4 = _run("l4", progs["l4"], ins4)
    out = np.zeros((P, 1), np.float32)
    for c in range(NCORES):
        out[c * PC:(c + 1) * PC, 0] = r4[c]["z"][s % 128, s // 128]

    tot = sum(v for v in LAST_EXEC_NS.values() if v)
    print(f"kernel launches ns: {LAST_EXEC_NS} total {tot}")
    return out


# revision 97
# speedup vs baseline: 1.0651x; 1.0651x over previous
"""Two-layer GAT (single-head, PyG-style) + link predictor on 8 TRN2 NeuronCores.

Strategy (memory-regime):
  - Nodes sharded 8-way (6250/core, 49 windows of 128 dst nodes); edges
    (incl. self-loops) assigned to the core owning their dst and sorted by
    dst, so edge-softmax and the weighted scatter-sum are core-local.
  - The halo exchange runs on the host between launches: per-edge source
    feature rows are pre-expanded into a sequential fp16 stream
    [128, T, cols] (slot (p,t) = edge s%128, s//128 within its window), so
    the device does only large contiguous DMAs - no indirect gathers.
  - Segment softmax + weighted scatter run as one-hot matmuls on the PE:
        psum[d, :] += sum_e p_e * [dst_e == d] * stream[e, :]
    with a constant 1.0 column in each stream row accumulating the softmax
    denominator. exp() needs no segment-max shift (logits are O(6) and the
    shift cancels in the ratio). Sel matrices for a whole window are built
    with two stacked DVE ops using stride-0 3D broadcast APs.
  - Launch fusion: L2 = agg1 + proj2 (PE-transpose of the aggregated
    window then W2 matmul), with es2/ed2 and the link-predictor partial
    dots (W2@wl0, W2@wl1 columns) folded into the projection. L3 = agg2
    emitting only per-node d0/d1 dots; L4 combines sigmoid(d0[m0]+d1[m1]+b).
  - All floating-point math happens on device; the host does index-space
    work only (partitioning, sorting, expansion, fp16 table assembly).

Launches: L1 proj1 -> L2 agg1+proj2 -> L3 agg2+dots -> L4 combine.
"""
import sys
import time
import types

import numpy as np

# Environments differ in whether antenv.axon_hooks (the NTFF profile hook
# bridge) exists; install a shim wired to the boot helper when it's missing
# so trace=True works everywhere.
try:
    import antenv.axon_hooks  # noqa: F401
except ImportError:
    _hooks = types.ModuleType("antenv.axon_hooks")
    _hooks._hook = None
    _hooks.set_axon_ntff_profile_hook = lambda h: setattr(_hooks, "_hook", h)
    _hooks.get_axon_ntff_profile_hook = lambda: _hooks._hook
    sys.modules["antenv.axon_hooks"] = _hooks
    try:
        from trn_agent_boot.trn_boot import _ntff_profile_via_ctypes

        _hk = _ntff_profile_via_ctypes("/opt/axon/libaxon_pjrt.so")
        if _hk is not None:
            _hooks.set_axon_ntff_profile_hook(_hk)
    except Exception:
        pass

import concourse.bass as bass  # noqa: F401  (AP helpers)
import concourse.mybir as mybir
import concourse.tile as tile
from concourse import bacc
from concourse.bass_utils import run_bass_kernel_spmd

F32 = mybir.dt.float32
F16 = mybir.dt.float16
I32 = mybir.dt.int32

NCORES = 8
N, F_IN, H, C = 50000, 128, 256, 1
P = 10000
NS = N // NCORES            # 6250 nodes per shard
W = (NS + 127) // 128       # 49 windows per shard
NSP = W * 128               # 6272 padded slots
NEG = -1.0e30               # pad-edge sentinel (exp -> exactly 0)
PC = P // NCORES            # 1250 mask pairs per core
PT = (PC + 127) // 128      # 10 tiles of pairs

LAST_EXEC_NS = {}           # launch name -> exec_time_ns (filled per kernel() call)
_PROG_CACHE = {}
CMP_GPSIMD = False          # gpsimd can't lower broadcast APs; keep cmp on DVE


# ----------------------------------------------------------------- host prep
def _prep_graph(edge_index, wd):
    """Edges (incl. self-loops) partitioned by dst core, sorted by dst,
    window-padded to a common per-window tile count across cores. Windows
    are wd dst nodes wide; edge slot s within window w is
    (p, t) = (s % 128, wstart[w] + s // 128)."""
    nw = NSP // wd
    src = np.concatenate(
        [np.asarray(edge_index[0], np.int64), np.arange(N, dtype=np.int64)]
    )
    dst = np.concatenate(
        [np.asarray(edge_index[1], np.int64), np.arange(N, dtype=np.int64)]
    )
    core = dst // NS
    dstloc = dst - core * NS
    win = dstloc // wd

    order = np.lexsort((dstloc, core))
    src, core, dstloc, win = src[order], core[order], dstloc[order], win[order]

    cnt = np.zeros((NCORES, nw), np.int64)
    np.add.at(cnt, (core, win), 1)
    wt = np.maximum(1, (cnt + 127) // 128).max(axis=0)
    T = int(wt.sum())
    wstart = np.concatenate([[0], np.cumsum(wt)]).astype(np.int64)

    gid = core * nw + win
    first = np.ones(len(gid), bool)
    first[1:] = gid[1:] != gid[:-1]
    gstart = np.flatnonzero(first)
    startmap = np.zeros(NCORES * nw, np.int64)
    startmap[gid[gstart]] = gstart
    rank = np.arange(len(gid)) - startmap[gid]

    tt = wstart[win] + (rank >> 7)
    pp = rank & 127

    srcs = np.zeros((NCORES, 128, T), np.int32)
    dstg = np.zeros((NCORES, 128, T), np.int32)
    dstf = np.full((NCORES, 128, T), -1.0, np.float32)
    pad = np.ones((NCORES, 128, T), bool)
    srcs[core, pp, tt] = src
    dstg[core, pp, tt] = dstloc + core * NS
    dstf[core, pp, tt] = (dstloc - win * wd).astype(np.float32)
    pad[core, pp, tt] = False
    return dict(srcs=srcs, dstg=dstg, dstf=dstf, pad=pad, wt=wt, T=T, wd=wd)


def _edge_inputs(es, ed, g, c):
    """Per-slot es[src], ed[dst] (f32), with pad slots set to the exp->0
    sentinel."""
    esx = es[g["srcs"][c]].astype(np.float32)
    edx = ed[g["dstg"][c]].astype(np.float32)
    m = g["pad"][c]
    esx[m] = NEG
    edx[m] = 0.0
    return esx, edx


def _balance_perm(edge_index):
    """Permutation old-node -> new-node that balances (in-degree + 1) across
    the 392 (core, 128-window) bins, so per-window tile counts are near the
    mean instead of the max. Within each bin, snake-order by degree so the
    two 64-wide halves stay balanced too. Pure index-space work."""
    import heapq

    deg = np.bincount(
        np.asarray(edge_index[1], np.int64), minlength=N
    ) + 1
    caps = []
    for c in range(NCORES):
        for w in range(W):
            caps.append(min(128, NS - 128 * w))
    members = [[] for _ in caps]
    heap = [(0, b) for b in range(len(caps))]
    heapq.heapify(heap)
    order = np.argsort(-deg, kind="stable")
    for v in order:
        while True:
            load, b = heapq.heappop(heap)
            if len(members[b]) < caps[b]:
                break
        members[b].append(v)
        if len(members[b]) < caps[b]:
            heapq.heappush(heap, (load + int(deg[v]), b))
    perm = np.empty(N, np.int64)
    for b, mem in enumerate(members):
        c, w = divmod(b, W)
        # alternate degree-sorted members between the two 64-wide halves
        snake = mem[0::2] + mem[1::2][::-1]
        base = c * NS + 128 * w
        for i, v in enumerate(snake):
            perm[v] = base + i
    return perm


def _rep(v, n=128):
    return np.ascontiguousarray(
        np.broadcast_to(np.asarray(v, np.float32), (n, len(v)))
    )


def _tile_xT(x):
    """[N, 128] f32 features -> per-core [128, W*128] f16 transposed
    feature block for the L1 matmul lhsT slices."""
    out = np.zeros((NCORES, 128, W * 128), np.float16)
    for c in range(NCORES):
        xs = np.zeros((NSP, F_IN), np.float16)
        xs[:NS] = x[c * NS:(c + 1) * NS]
        out[c] = xs.T
    return out


# ------------------------------------------------------------- bass programs
def _build_p1(bias_zero):
    """L1: psum = xT.T @ [W1 | W1@a_s1 | W1@a_d1] per window; one fp16
    cast (+b1 fold) of the full 258-col psum per window -> h1e. Chunked
    input/output DMAs to stay off the HWDGE dispatch serialization. When
    b1 is all zero the psum->stage ops are plain casts split across the
    Act and DVE engines."""
    nc = bacc.Bacc(num_devices=NCORES)
    xT = nc.dram_tensor("xT", [128, W * 128], F16, kind="ExternalInput").ap()
    Wm = nc.dram_tensor("Wm", [F_IN, H], F16, kind="ExternalInput").ap()
    asr = nc.dram_tensor("asr", [128, H], F32, kind="ExternalInput").ap()
    adr = nc.dram_tensor("adr", [128, H], F32, kind="ExternalInput").ap()
    b1r = nc.dram_tensor("b1r", [128, H], F32, kind="ExternalInput").ap()
    h1e = nc.dram_tensor(
        "h1e", [128, W * (H + 2)], F16, kind="ExternalOutput"
    ).ap()

    with tile.TileContext(nc) as tc:
        with (
            tc.tile_pool(name="const", bufs=1) as cpool,
            tc.tile_pool(name="ps", bufs=4, space="PSUM") as pspool,
            tc.tile_pool(name="sc", bufs=2) as scpool,
        ):
            asb = cpool.tile([128, H], F32)
            nc.gpsimd.dma_start(out=asb[:], in_=asr[:])
            adb = cpool.tile([128, H], F32)
            nc.gpsimd.dma_start(out=adb[:], in_=adr[:])
            # b1 is folded into the message rows here: softmax weights sum
            # to one, so agg(h1 + b1) == agg(h1) + b1 downstream.
            b1x = cpool.tile([128, H + 2], F32)
            nc.vector.memset(b1x[:, H:H + 2], 0.0)
            nc.gpsimd.dma_start(out=b1x[:, 0:H], in_=b1r[:])
            xts = cpool.tile([128, W * 128], F16)
            nchunk = 7
            for k in range(nchunk):
                nc.sync.dma_start(
                    out=xts[:, k * W * 128 // nchunk:(k + 1) * W * 128 // nchunk],
                    in_=xT[:, k * W * 128 // nchunk:(k + 1) * W * 128 // nchunk],
                )
            waug = cpool.tile([128, H + 2], F16)
            nc.sync.dma_start(out=waug[:, 0:H], in_=Wm[:])
            w32 = cpool.tile([128, H], F32)
            nc.vector.tensor_copy(out=w32[:], in_=waug[:, 0:H])
            for j, vb in enumerate((asb, adb)):
                scr = scpool.tile([128, H], F32, tag="scr")
                nc.vector.tensor_tensor(
                    out=scr[:], in0=w32[:], in1=vb[:], op=mybir.AluOpType.mult
                )
                col = scpool.tile([128, 1], F32, tag="col")
                nc.vector.reduce_sum(
                    out=col[:], in_=scr[:], axis=mybir.AxisListType.X
                )
                nc.vector.tensor_copy(out=waug[:, H + j:H + j + 1], in_=col[:])

            stage = cpool.tile([128, W * (H + 2)], F16)
            WC = (W + nchunk - 1) // nchunk
            for w in range(W):
                ps = pspool.tile([128, H + 2], F32, space="PSUM")
                nc.tensor.matmul(
                    out=ps[:], lhsT=xts[:, 128 * w:128 * (w + 1)],
                    rhs=waug[:], start=True, stop=True,
                )
                dst = stage[:, w * (H + 2):(w + 1) * (H + 2)]
                if bias_zero and w % 2 == 0:
                    nc.scalar.copy(out=dst, in_=ps[:])
                elif bias_zero:
                    nc.vector.tensor_copy(out=dst, in_=ps[:])
                else:
                    nc.vector.tensor_tensor(
                        out=dst, in0=ps[:], in1=b1x[:],
                        op=mybir.AluOpType.add,
                    )
                if w % WC == WC - 1 or w == W - 1:
                    lo = (w // WC) * WC * (H + 2)
                    nc.sync.dma_start(
                        out=h1e[:, lo:(w + 1) * (H + 2)],
                        in_=stage[:, lo:(w + 1) * (H + 2)],
                    )
    nc.compile()
    return nc


def _build_agg(wt, cols, fuse_proj, wd):
    """Aggregation launch (one GAT layer).

    cols = stream row width (incl. trailing 1.0 denominator column and, for
    L3, the w0/w1 dot columns). Per window: one stream-slab DMA, a 2-op
    stacked sel build, wt[w] one-hot matmuls into psum, then either
      fuse_proj=True  (L2): normalize+bias+relu -> PE transpose -> W2aug
                      matmul -> h2e [NSP, 132] fp16 out
      fuse_proj=False (L3): d0/d1 = psum dot cols * rec + (b2.wl) -> d01.
    """
    T = int(sum(wt))
    npair = 128 // wd
    wtp = [
        sum(int(wt[npair * pw + s]) for s in range(npair))
        for pw in range(W)
    ]
    WTP = max(wtp)
    nc = bacc.Bacc(num_devices=NCORES)
    stream = nc.dram_tensor(
        "stream", [128, T * cols], F16, kind="ExternalInput"
    ).ap()
    dstf = nc.dram_tensor("dstf", [128, T], F16, kind="ExternalInput").ap()
    esx = nc.dram_tensor("esx", [128, T], F32, kind="ExternalInput").ap()
    edx = nc.dram_tensor("edx", [128, T], F32, kind="ExternalInput").ap()
    iota3 = nc.dram_tensor(
        "iota3", [128, wd, WTP], F16, kind="ExternalInput"
    ).ap()
    if fuse_proj:
        w2m = nc.dram_tensor("w2m", [H, F_IN], F16, kind="ExternalInput").ap()
        vr = [
            nc.dram_tensor(nm, [128, F_IN], F32, kind="ExternalInput").ap()
            for nm in ("as2r", "ad2r", "wl0r", "wl1r")
        ]
        idn = nc.dram_tensor("idn", [128, 128], F16, kind="ExternalInput").ap()
        h2e = nc.dram_tensor(
            "h2e", [128, W * (F_IN + 4)], F16, kind="ExternalOutput"
        ).ap()
    else:
        b2r = nc.dram_tensor("b2r", [128, F_IN], F32, kind="ExternalInput").ap()
        wl0r = nc.dram_tensor("wl0r", [128, F_IN], F32, kind="ExternalInput").ap()
        wl1r = nc.dram_tensor("wl1r", [128, F_IN], F32, kind="ExternalInput").ap()
        d01 = nc.dram_tensor("d01", [128, 2 * W], F32, kind="ExternalOutput").ap()

    with tile.TileContext(nc) as tc:
        with (
            tc.tile_pool(name="const", bufs=1) as cpool,
            tc.tile_pool(name="slab", bufs=6) as spool,
            tc.tile_pool(name="cmp", bufs=4) as cmppool,
            tc.tile_pool(name="sel", bufs=4) as selpool,
            tc.tile_pool(name="ep", bufs=3) as eppool,
            tc.tile_pool(name="o", bufs=3) as opool,
            tc.tile_pool(name="ps", bufs=4, space="PSUM") as pspool,
            tc.tile_pool(name="pt", bufs=2, space="PSUM") as ptpool,
            tc.tile_pool(name="p2", bufs=2, space="PSUM") as p2pool,
        ):
            # const inputs load via the idle gpsimd queue so the Sync queue
            # can start dispatching stream slabs immediately
            dsts = cpool.tile([128, T], F16)
            nc.gpsimd.dma_start(out=dsts[:], in_=dstf[:])
            esxs = cpool.tile([128, T], F32)
            nc.gpsimd.dma_start(out=esxs[:], in_=esx[:])
            edxs = cpool.tile([128, T], F32)
            nc.gpsimd.dma_start(out=edxs[:], in_=edx[:])
            io3 = cpool.tile([128, wd, WTP], F16)
            nc.gpsimd.dma_start(out=io3[:], in_=iota3[:])

            if fuse_proj:
                ids = cpool.tile([128, 128], F16)
                nc.gpsimd.dma_start(out=ids[:], in_=idn[:])
                vs = []
                for k, ap_ in enumerate(vr):
                    t_ = cpool.tile([128, F_IN], F32, tag=f"v{k}")
                    nc.gpsimd.dma_start(out=t_[:], in_=ap_[:])
                    vs.append(t_)
                w2aug = []
                for k in range(2):
                    wk = cpool.tile([128, F_IN + 4], F16, tag=f"w2a{k}")
                    nc.gpsimd.dma_start(
                        out=wk[:, 0:F_IN], in_=w2m[128 * k:128 * (k + 1), :]
                    )
                    wk32 = cpool.tile([128, F_IN], F32, tag=f"w232{k}")
                    nc.vector.tensor_copy(out=wk32[:], in_=wk[:, 0:F_IN])
                    for j, vb in enumerate(vs):
                        scr = cpool.tile([128, F_IN], F32, tag="fscr")
                        nc.vector.tensor_tensor(
                            out=scr[:], in0=wk32[:], in1=vb[:],
                            op=mybir.AluOpType.mult,
                        )
                        col = cpool.tile([128, 1], F32, tag="fcol")
                        nc.vector.reduce_sum(
                            out=col[:], in_=scr[:], axis=mybir.AxisListType.X
                        )
                        nc.vector.tensor_copy(
                            out=wk[:, F_IN + j:F_IN + j + 1], in_=col[:]
                        )
                    w2aug.append(wk)
            else:
                b2s = cpool.tile([128, F_IN], F32)
                nc.gpsimd.dma_start(out=b2s[:], in_=b2r[:])
                wl0s = cpool.tile([128, F_IN], F32)
                nc.gpsimd.dma_start(out=wl0s[:], in_=wl0r[:])
                wl1s = cpool.tile([128, F_IN], F32)
                nc.gpsimd.dma_start(out=wl1s[:], in_=wl1r[:])
                cc = cpool.tile([128, 2], F32)
                for j, vb in enumerate((wl0s, wl1s)):
                    scr = cpool.tile([128, F_IN], F32, tag="cscr")
                    nc.vector.tensor_tensor(
                        out=scr[:], in0=b2s[:], in1=vb[:],
                        op=mybir.AluOpType.mult,
                    )
                    nc.vector.reduce_sum(
                        out=cc[:, j:j + 1], in_=scr[:], axis=mybir.AxisListType.X
                    )
                d01s = cpool.tile([128, 2 * W], F32)

            # softmax numerators p = exp(leaky_relu(es+ed, 0.2)) in fp16
            lg = cpool.tile([128, T], F32)
            nc.vector.tensor_tensor(
                out=lg[:], in0=esxs[:], in1=edxs[:], op=mybir.AluOpType.add
            )
            lg2 = cpool.tile([128, T], F32)
            nc.vector.tensor_scalar_mul(out=lg2[:], in0=lg[:], scalar1=0.2)
            nc.vector.tensor_tensor(
                out=lg[:], in0=lg[:], in1=lg2[:], op=mybir.AluOpType.max
            )
            p16 = cpool.tile([128, T], F16)
            nc.scalar.activation(
                out=p16[:], in_=lg[:], func=mybir.ActivationFunctionType.Exp
            )

            if fuse_proj:
                stage = cpool.tile([128, W * (F_IN + 4)], F16)
            dcol = cols - 1 if fuse_proj else F_IN

            def epilogue(ps, w):
                rec = eppool.tile([128, 1], F32, tag="rec")
                nc.vector.reciprocal(rec[:], ps[:, dcol:dcol + 1])
                if fuse_proj:
                    # b1 is pre-folded into the message rows; normalize and
                    # rectify in one Act op: relu(agg * (1/den)).
                    h1r = eppool.tile([128, H], F16, tag="h1r")
                    nc.scalar.activation(
                        out=h1r[:], in_=ps[:, 0:H],
                        func=mybir.ActivationFunctionType.Relu,
                        scale=rec[:, :1],
                    )
                    xt = eppool.tile([128, H], F16, tag="xt")
                    psT = ptpool.tile([128, H], F16, space="PSUM")
                    for ck in range(2):
                        nc.tensor.transpose(
                            out=psT[:, 128 * ck:128 * (ck + 1)],
                            in_=h1r[:, 128 * ck:128 * (ck + 1)],
                            identity=ids[:],
                        )
                    nc.scalar.copy(out=xt[:], in_=psT[:])
                    ps2 = p2pool.tile([128, F_IN + 4], F32, space="PSUM")
                    nc.tensor.matmul(
                        out=ps2[:], lhsT=xt[:, 0:128], rhs=w2aug[0][:],
                        start=True, stop=False,
                    )
                    nc.tensor.matmul(
                        out=ps2[:], lhsT=xt[:, 128:256], rhs=w2aug[1][:],
                        start=False, stop=True,
                    )
                    nc.scalar.copy(
                        out=stage[:, w * (F_IN + 4):(w + 1) * (F_IN + 4)],
                        in_=ps2[:],
                    )
                    if w % 7 == 6 or w == W - 1:
                        lo = (w // 7) * 7 * (F_IN + 4)
                        nc.sync.dma_start(
                            out=h2e[:, lo:(w + 1) * (F_IN + 4)],
                            in_=stage[:, lo:(w + 1) * (F_IN + 4)],
                        )
                else:
                    nc.vector.scalar_tensor_tensor(
                        out=d01s[:, 2 * w:2 * w + 2],
                        in0=ps[:, F_IN + 1:F_IN + 3], scalar=rec[:, :1],
                        in1=cc[:], op0=mybir.AluOpType.mult,
                        op1=mybir.AluOpType.add,
                    )

            t0 = 0
            pending = None
            for pw in range(W):
                wtpg = wtp[pw]
                # one slab DMA per 128-node group of WD-wide windows
                slab = spool.tile([128, WTP * cols], F16)
                nc.sync.dma_start(
                    out=slab[:, 0:wtpg * cols],
                    in_=stream[:, t0 * cols:(t0 + wtpg) * cols],
                )
                # sel layout [slot, dst, tile]: per-(slot,tile) operands
                # broadcast on the MIDDLE dim, keeping innermost stride 1 so
                # the DVE 2x perf mode stays eligible. dstf is window-local,
                # so one op-pair covers every sub-window of the group.
                cmp3 = cmppool.tile([128, wd, WTP], F16)
                nc.vector.tensor_tensor(
                    out=cmp3[:, :, 0:wtpg], in0=io3[:, :, 0:wtpg],
                    in1=dsts[:, t0:t0 + wtpg].unsqueeze(1)
                        .broadcast_to([128, wd, wtpg]),
                    op=mybir.AluOpType.is_equal,
                )
                sel3 = selpool.tile([128, wd, WTP], F16)
                nc.vector.tensor_tensor(
                    out=sel3[:, :, 0:wtpg], in0=cmp3[:, :, 0:wtpg],
                    in1=p16[:, t0:t0 + wtpg].unsqueeze(1)
                        .broadcast_to([128, wd, wtpg]),
                    op=mybir.AluOpType.mult,
                )
                ps = pspool.tile([128, cols], F32, space="PSUM")
                tp = 0
                for sub in range(npair):
                    wtw = int(wt[npair * pw + sub])
                    for t in range(wtw):
                        nc.tensor.matmul(
                            out=ps[wd * sub:wd * (sub + 1), :],
                            lhsT=sel3[:, :, tp + t],
                            rhs=slab[:, (tp + t) * cols:(tp + t + 1) * cols],
                            start=(t == 0), stop=(t == wtw - 1),
                        )
                    tp += wtw
                # emit the previous window's epilogue AFTER this window's
                # cmp/sel, so the in-order DVE queue never head-of-line
                # blocks on reciprocal waiting for the matmuls to drain
                if pending is not None:
                    epilogue(*pending)
                pending = (ps, pw)
                t0 += wtp[pw]
            epilogue(*pending)
            if not fuse_proj:
                nc.sync.dma_start(out=d01[:], in_=d01s[:])
    nc.compile()
    return nc


def _build_comb():
    """L4: z = sigmoid(d0[m0] + d1[m1] + bl) for PC pairs per core."""
    nc = bacc.Bacc(num_devices=NCORES)
    d0x = nc.dram_tensor("d0x", [128, PT], F32, kind="ExternalInput").ap()
    d1x = nc.dram_tensor("d1x", [128, PT], F32, kind="ExternalInput").ap()
    blr = nc.dram_tensor("blr", [128, 1], F32, kind="ExternalInput").ap()
    z = nc.dram_tensor("z", [128, PT], F32, kind="ExternalOutput").ap()

    with tile.TileContext(nc) as tc:
        with tc.tile_pool(name="p", bufs=1) as pool:
            d0s = pool.tile([128, PT], F32)
            nc.sync.dma_start(out=d0s[:], in_=d0x[:])
            d1s = pool.tile([128, PT], F32)
            nc.sync.dma_start(out=d1s[:], in_=d1x[:])
            bls = pool.tile([128, 1], F32)
            nc.sync.dma_start(out=bls[:], in_=blr[:])
            ss = pool.tile([128, PT], F32)
            nc.vector.tensor_tensor(
                out=ss[:], in0=d0s[:], in1=d1s[:], op=mybir.AluOpType.add
            )
            zs = pool.tile([128, PT], F32)
            nc.scalar.activation(
                out=zs[:], in_=ss[:],
                func=mybir.ActivationFunctionType.Sigmoid, bias=bls[:, :1],
            )
            nc.sync.dma_start(out=z[:], in_=zs[:])
    nc.compile()
    return nc


def _run(name, nc, in_maps, trace=True):
    last = None
    for attempt in range(3):
        try:
            res = run_bass_kernel_spmd(
                nc, in_maps, core_ids=list(range(NCORES)),
                trace=trace and attempt < 2,
            )
            LAST_EXEC_NS[name] = res.exec_time_ns
            return res.results
        except Exception as e:  # wedged-device retry (clears on re-attempt)
            last = e
            time.sleep(5)
    raise last


# ------------------------------------------------------------------- kernel
def kernel(features, edge_index, mask, W1, a_src1, a_dst1, b1, W2, a_src2,
           a_dst2, b2, Wl, bl):
    features = np.asarray(features, np.float32)
    edge_index = np.asarray(edge_index, np.int32)
    mask = np.asarray(mask, np.int32)
    W1, W2, Wl = (np.asarray(a, np.float32) for a in (W1, W2, Wl))
    a_src1, a_dst1, b1 = (np.asarray(a, np.float32) for a in (a_src1, a_dst1, b1))
    a_src2, a_dst2, b2 = (np.asarray(a, np.float32) for a in (a_src2, a_dst2, b2))
    bl = np.asarray(bl, np.float32)

    # degree-balanced node relabeling (transparent: mask rows keep order)
    perm = _balance_perm(edge_index)
    features = features[np.argsort(perm)]
    edge_index = perm[edge_index].astype(np.int32)
    mask = perm[mask].astype(np.int32)

    g2 = _prep_graph(edge_index, 128)   # L2: wide windows (moving cols dominate)
    g3 = _prep_graph(edge_index, 64)    # L3: narrow windows (sel build dominates)

    def _iota3(g):
        wd = g["wd"]
        npair = 128 // wd
        wtp = g["wt"].reshape(-1, npair).sum(axis=1)
        return np.ascontiguousarray(np.broadcast_to(
            np.arange(wd, dtype=np.float16)[None, :, None],
            (128, wd, int(max(wtp))),
        ))

    iota2, iota3_ = _iota3(g2), _iota3(g3)
    idn = np.eye(128, dtype=np.float16)
    dstf2 = g2["dstf"].astype(np.float16)
    dstf3 = g3["dstf"].astype(np.float16)

    b1zero = not np.any(b1)
    key = (tuple(int(x) for x in g2["wt"]), tuple(int(x) for x in g3["wt"]),
           b1zero)
    if key not in _PROG_CACHE:
        _PROG_CACHE[key] = dict(
            p1=_build_p1(b1zero),
            l2=_build_agg(g2["wt"], H + 1, fuse_proj=True, wd=128),
            l3=_build_agg(g3["wt"], F_IN + 3, fuse_proj=False, wd=64),
            l4=_build_comb(),
        )
    progs = _PROG_CACHE[key]

    # ---- L1: h1e = [X@W1 | es1 | ed1] (sharded)
    xT = _tile_xT(features)
    W1h = W1.astype(np.float16)
    b1r = _rep(b1)
    r1 = _run("p1", progs["p1"], [
        dict(xT=xT[c], Wm=W1h, asr=_rep(a_src1), adr=_rep(a_dst1), b1r=b1r)
        for c in range(NCORES)
    ])
    H1 = np.concatenate([
        r1[c]["h1e"].reshape(128, W, H + 2).transpose(1, 0, 2)
        .reshape(NSP, H + 2)[:NS]
        for c in range(NCORES)
    ])  # [N, 258] f16
    es1 = H1[:, H].astype(np.float32)
    ed1 = H1[:, H + 1].astype(np.float32)
    table1 = np.empty((N, H + 1), np.float16)
    table1[:, :H] = H1[:, :H]
    table1[:, H] = 1.0

    # ---- L2: aggregate layer 1, project through W2aug
    as2r, ad2r = _rep(a_src2), _rep(a_dst2)
    wl0r, wl1r = _rep(Wl[:F_IN, 0]), _rep(Wl[F_IN:, 0])
    W2h = W2.astype(np.float16)
    ins2 = []
    for c in range(NCORES):
        esx, edx = _edge_inputs(es1, ed1, g2, c)
        strm = table1[g2["srcs"][c]].reshape(128, g2["T"] * (H + 1))
        ins2.append(dict(stream=strm, dstf=dstf2[c], esx=esx, edx=edx,
                         iota3=iota2, w2m=W2h, as2r=as2r, ad2r=ad2r,
                         wl0r=wl0r, wl1r=wl1r, idn=idn))
    r2 = _run("l2", progs["l2"], ins2)
    H2 = np.concatenate([
        r2[c]["h2e"].reshape(128, W, F_IN + 4).transpose(1, 0, 2)
        .reshape(NSP, F_IN + 4)[:NS]
        for c in range(NCORES)
    ])  # [N, 132] f16
    es2 = H2[:, F_IN].astype(np.float32)
    ed2 = H2[:, F_IN + 1].astype(np.float32)
    table2 = np.empty((N, F_IN + 3), np.float16)
    table2[:, :F_IN] = H2[:, :F_IN]
    table2[:, F_IN] = 1.0
    table2[:, F_IN + 1:F_IN + 3] = H2[:, F_IN + 2:F_IN + 4]

    # ---- L3: aggregate layer 2 -> per-node link dots d0, d1
    b2r = _rep(b2)
    ins3 = []
    for c in range(NCORES):
        esx, edx = _edge_inputs(es2, ed2, g3, c)
        strm = table2[g3["srcs"][c]].reshape(128, g3["T"] * (F_IN + 3))
        ins3.append(dict(stream=strm, dstf=dstf3[c], esx=esx, edx=edx,
                         iota3=iota3_, b2r=b2r, wl0r=wl0r, wl1r=wl1r))
    r3 = _run("l3", progs["l3"], ins3)
    d0g = np.concatenate(
        [r3[c]["d01"][:, 0::2].T.ravel()[:NS] for c in range(NCORES)]
    )
    d1g = np.concatenate(
        [r3[c]["d01"][:, 1::2].T.ravel()[:NS] for c in range(NCORES)]
    )

    # ---- L4: z = sigmoid(d0[m0] + d1[m1] + bl)
    mT = mask.T
    blr = np.full((128, 1), float(bl[0]), np.float32)
    s = np.arange(PC)
    ins4 = []
    for c in range(NCORES):
        d0x = np.zeros((128, PT), np.float32)
        d1x = np.zeros((128, PT), np.float32)
        d0x[s % 128, s // 128] = d0g[mT[0][c * PC:(c + 1) * PC]]
        d1x[s % 128, s // 128] = d1g[mT[1][c * PC:(c + 1) * PC]]
        ins4.append(dict(d0x=d0x, d1x=d1x, blr=blr))
    r4 = _run("l4", progs["l4"], ins4)
    out = np.zeros((P, 1), np.float32)
    for c in range(NCORES):
        out[c * PC:(c + 1) * PC, 0] = r4[c]["z"][s % 128, s // 128]

    tot = sum(v for v in LAST_EXEC_NS.values() if v)
    print(f"kernel launches ns: {LAST_EXEC_NS} total {tot}")
    return out
